# revision 9
# baseline (speedup 1.0000x reference)
"""Mixture-of-Experts (top-1 routing) Trainium2 kernel.

Strategy (expert-parallel, per sharding hint):
 - Router (softmax / argmax / top-prob) evaluated on host — 8192x8, i.e.
   0.002% of the FLOPs; its cost is dispatch bookkeeping.
 - Tokens are dispatched to the core owning their expert: core e receives
   the tokens routed to expert e (transposed, pre-scaled by the gate
   probability, padded to capacity C), plus W[e] and b[e].
 - Each core runs a dense [C,1024] @ [1024,1024] GEMM on the TensorEngine
   in float32r (full-rate fp32, ~1.5e-4 max rel err).  PSUM eviction fuses
   the bias: out = (bias * top_p) + psum in one DVE op per tile.
 - Host scatters the compact per-expert outputs back to token order
   (the "second all-to-all" / unshard step).
"""

import numpy as np

T, H, E = 8192, 1024, 8
N_CORES = 8
P = 128
KT = H // P          # 8 contraction tiles
NFREE = 512          # matmul moving free dim (one PSUM bank of fp32)
NT = H // NFREE      # 2 output column tiles

_BUILD_CACHE = {}


def _build(C):
    """Build the SPMD Bass module for per-core token capacity C (multiple of 128)."""
    import concourse.mybir as mybir
    import concourse.tile as tile
    from concourse import bacc

    MT = C // P
    DT = mybir.dt.float16    # half-precision I/O, full-rate matmul
    F32 = mybir.dt.float32
    F16 = mybir.dt.float16
    ALU = mybir.AluOpType

    nc = bacc.Bacc("TRN2", target_bir_lowering=False, debug=False,
                   num_devices=N_CORES)

    xt_d = nc.dram_tensor("xt", [KT, P, C], DT, kind="ExternalInput").ap()
    w_d = nc.dram_tensor("w", [KT, P, H], DT, kind="ExternalInput").ap()
    bias_d = nc.dram_tensor("bias", [P, H], F16, kind="ExternalInput").ap()
    scale_d = nc.dram_tensor("scale", [P, MT], F32, kind="ExternalInput").ap()
    out_d = nc.dram_tensor("out", [MT, P, H], F16, kind="ExternalOutput").ap()

    CH = 4  # m-tiles per chunk (4m x 2n = 8 PSUM banks)
    m_chunks = [list(range(s, min(s + CH, MT))) for s in range(0, MT, CH)]
    # split the final chunk into single m-tiles so the kernel tail
    # (eviction + out-DMA after the last matmul) is as short as possible
    if len(m_chunks) > 1 and len(m_chunks[-1]) > 1:
        last = m_chunks.pop()
        m_chunks.extend([m] for m in last)

    with tile.TileContext(nc) as tc:
        with (
            tc.tile_pool(name="ins", bufs=1) as ins,
            tc.tile_pool(name="psum", bufs=1, space="PSUM") as psum_pool,
            tc.tile_pool(name="outp", bufs=4) as outp,
        ):
            xt_sb = [ins.tile([P, C], DT, name=f"xt{k}") for k in range(KT)]
            w_sb = [ins.tile([P, H], DT, name=f"w{k}") for k in range(KT)]
            bias_sb = ins.tile([P, H], F16, name="bias")
            scale_sb = ins.tile([P, MT], F32, name="scale")

            # PE warm-up: 8 dummy matmuls on a zeroed tile run during the
            # DMA head phase so the HAM clock-gate opens (1.2->2.4 GHz)
            # before the first real matmul issues.  The warm-up PSUM tile
            # shares the ps0_0 slot: its last write completes long before
            # chunk 0's data arrives, so the WAW hand-off is free.
            wz = ins.tile([P, P + NFREE], DT, name="wz")
            nc.gpsimd.memset(wz[:], 0)
            warm_ps = psum_pool.tile([P, NFREE], F32, name="ps0_0")
            for _ in range(8):
                nc.tensor.matmul(warm_ps[:], wz[:, :P], wz[:, P:],
                                 start=True, stop=True)

            # tiny scale/bias inputs ride the SWDGE GpSimd queue, keeping
            # both HWDGE queues free for the k-major xt / w streams
            nc.gpsimd.dma_start(scale_sb[:], scale_d[:])
            nc.gpsimd.dma_start(bias_sb[:], bias_d[:])
            for k in range(KT):
                nc.sync.dma_start(xt_sb[k][:], xt_d[k])
                nc.scalar.dma_start(w_sb[k][:], w_d[k])

            for chunk in m_chunks:
                ps = {}
                for m in chunk:
                    for n in range(NT):
                        ps[m, n] = psum_pool.tile([P, NFREE], F32,
                                                  name=f"ps{m % CH}_{n}")
                for k in range(KT):
                    for m in chunk:
                        for n in range(NT):
                            nc.tensor.matmul(
                                ps[m, n][:],
                                xt_sb[k][:, m * P:(m + 1) * P],
                                w_sb[k][:, n * NFREE:(n + 1) * NFREE],
                                start=(k == 0), stop=(k == KT - 1),
                            )
                for mi, m in enumerate(chunk):
                    t = outp.tile([P, H], F16, name="osb")
                    for n in range(NT):
                        nsl = slice(n * NFREE, (n + 1) * NFREE)
                        # out = bias * top_p + psum   (single DVE op)
                        nc.vector.scalar_tensor_tensor(
                            t[:, nsl], bias_sb[:, nsl],
                            scale_sb[:, m:m + 1], ps[m, n][:],
                            op0=ALU.mult, op1=ALU.add,
                        )
                        if len(chunk) == 1:
                            # tail chunks: ship each half as soon as its
                            # eviction lands
                            eng = nc.sync if n == 0 else nc.scalar
                            eng.dma_start(out_d[m][:, nsl], t[:, nsl])
                    if len(chunk) > 1:
                        eng = nc.sync if mi % 2 == 0 else nc.scalar
                        eng.dma_start(out_d[m], t[:])

    nc.compile()
    return nc


def kernel(input, gate, W, b):
    from concourse import bass_utils

    input = np.ascontiguousarray(input, dtype=np.float32)
    gate = np.ascontiguousarray(gate, dtype=np.float32)
    W = np.ascontiguousarray(W, dtype=np.float32)
    b = np.ascontiguousarray(b, dtype=np.float32)

    # ---- router (host): top-1 expert + its softmax probability ----
    g = gate.astype(np.float64)
    gm = g.max(axis=1, keepdims=True)
    top_p = (1.0 / np.exp(g - gm).sum(axis=1)).astype(np.float32)
    e_t = np.argmax(gate, axis=1)

    counts = np.bincount(e_t, minlength=E)
    order = np.argsort(e_t, kind="stable")
    starts = np.zeros(E + 1, dtype=np.int64)
    np.cumsum(counts, out=starts[1:])

    C = max(P, int(-(-counts.max() // P)) * P)
    MT = C // P

    if C not in _BUILD_CACHE:
        _BUILD_CACHE[C] = _build(C)
    nc = _BUILD_CACHE[C]

    in_maps = []
    ids_per_e = []
    for e in range(E):
        ids = order[starts[e]:starts[e + 1]]
        ids_per_e.append(ids)
        n_e = len(ids)

        xt = np.zeros((KT, P, C), dtype=np.float16)
        # tokens pre-scaled by their gate probability
        xt.reshape(H, C)[:, :n_e] = (input[ids].T * top_p[ids][None, :]).astype(np.float16)

        scale = np.zeros((MT, P), dtype=np.float32)
        scale.reshape(C)[:n_e] = top_p[ids]
        scale = np.ascontiguousarray(scale.T)

        in_maps.append({
            "xt": xt,
            "w": W[e].astype(np.float16).reshape(KT, P, H),
            "bias": np.ascontiguousarray(np.broadcast_to(b[e].astype(np.float16), (P, H))),
            "scale": scale,
        })

    res = bass_utils.run_bass_kernel_spmd(nc, in_maps,
                                          core_ids=list(range(N_CORES)))

    out = np.empty((T, H), dtype=np.float32)
    for e in range(E):
        ids = ids_per_e[e]
        out[ids] = res.results[e]["out"].reshape(C, H)[:len(ids)].astype(np.float32)
    return out


# revision 10
# speedup vs baseline: 1.0404x; 1.0404x over previous
"""Mixture-of-Experts (top-1 routing) Trainium2 kernel.

Strategy (expert-parallel, per sharding hint):
 - Router (softmax / argmax / top-prob) evaluated on host — 8192x8, i.e.
   0.002% of the FLOPs; its cost is dispatch bookkeeping.
 - Tokens are dispatched to the core owning their expert: core e receives
   the tokens routed to expert e (transposed, pre-scaled by the gate
   probability, padded to capacity C), plus W[e] and b[e].
 - Each core runs a dense [C,1024] @ [1024,1024] GEMM on the TensorEngine
   with fp16 operands and fp32 PSUM accumulation (~4.5e-4 max rel err
   end-to-end).  PSUM eviction fuses the bias in a single DVE op per
   tile: out = (bias * top_p) + psum.
 - 8 dummy matmuls on zeroed tiles run during the DMA head phase to open
   the PE HAM clock gate (1.2 -> 2.4 GHz) before real work arrives.
 - Host scatters the compact per-expert outputs back to token order
   (the "second all-to-all" / unshard step).
"""

import numpy as np

T, H, E = 8192, 1024, 8
N_CORES = 8
P = 128
KT = H // P          # 8 contraction tiles
NFREE = 512          # matmul moving free dim (one PSUM bank of fp32)
NT = H // NFREE      # 2 output column tiles

_BUILD_CACHE = {}


def _build(C):
    """Build the SPMD Bass module for per-core token capacity C (multiple of 128)."""
    import concourse.mybir as mybir
    import concourse.tile as tile
    from concourse import bacc

    MT = C // P
    DT = mybir.dt.float16    # half-precision I/O, full-rate matmul
    F32 = mybir.dt.float32
    F16 = mybir.dt.float16
    ALU = mybir.AluOpType

    nc = bacc.Bacc("TRN2", target_bir_lowering=False, debug=False,
                   num_devices=N_CORES)

    xt_d = nc.dram_tensor("xt", [KT, P, C], DT, kind="ExternalInput").ap()
    w_d = nc.dram_tensor("w", [KT, P, H], DT, kind="ExternalInput").ap()
    bias_d = nc.dram_tensor("bias", [P, H], F16, kind="ExternalInput").ap()
    scale_d = nc.dram_tensor("scale", [P, MT], F32, kind="ExternalInput").ap()
    out_d = nc.dram_tensor("out", [MT, P, H], F16, kind="ExternalOutput").ap()

    CH = 4  # m-tiles per chunk (4m x 2n = 8 PSUM banks)
    m_chunks = [list(range(s, min(s + CH, MT))) for s in range(0, MT, CH)]
    # split the final chunk into single m-tiles so the kernel tail
    # (eviction + out-DMA after the last matmul) is as short as possible
    if len(m_chunks) > 1 and len(m_chunks[-1]) > 1:
        last = m_chunks.pop()
        m_chunks.extend([m] for m in last)

    with tile.TileContext(nc) as tc:
        with (
            tc.tile_pool(name="ins", bufs=1) as ins,
            tc.tile_pool(name="psum", bufs=1, space="PSUM") as psum_pool,
            tc.tile_pool(name="outp", bufs=4) as outp,
        ):
            xt_sb = [ins.tile([P, C], DT, name=f"xt{k}") for k in range(KT)]
            w_sb = [ins.tile([P, H], DT, name=f"w{k}") for k in range(KT)]
            bias_sb = ins.tile([P, H], F16, name="bias")
            scale_sb = ins.tile([P, MT], F32, name="scale")

            # PE warm-up: 8 dummy matmuls on a zeroed tile run during the
            # DMA head phase so the HAM clock-gate opens (1.2->2.4 GHz)
            # before the first real matmul issues.  The warm-up PSUM tile
            # shares the ps0_0 slot: its last write completes long before
            # chunk 0's data arrives, so the WAW hand-off is free.
            wz = ins.tile([P, P + NFREE], DT, name="wz")
            nc.gpsimd.memset(wz[:], 0)
            warm_ps = psum_pool.tile([P, NFREE], F32, name="ps0_0")
            for _ in range(8):
                nc.tensor.matmul(warm_ps[:], wz[:, :P], wz[:, P:],
                                 start=True, stop=True)

            # tiny scale/bias inputs ride the SWDGE GpSimd queue, keeping
            # both HWDGE queues free for the k-major xt / w streams
            nc.gpsimd.dma_start(scale_sb[:], scale_d[:])
            nc.gpsimd.dma_start(bias_sb[:], bias_d[:])
            for k in range(KT):
                nc.sync.dma_start(xt_sb[k][:], xt_d[k])
                nc.scalar.dma_start(w_sb[k][:], w_d[k])

            for chunk in m_chunks:
                ps = {}
                for m in chunk:
                    for n in range(NT):
                        ps[m, n] = psum_pool.tile([P, NFREE], F32,
                                                  name=f"ps{m % CH}_{n}")
                for k in range(KT):
                    for m in chunk:
                        for n in range(NT):
                            nc.tensor.matmul(
                                ps[m, n][:],
                                xt_sb[k][:, m * P:(m + 1) * P],
                                w_sb[k][:, n * NFREE:(n + 1) * NFREE],
                                start=(k == 0), stop=(k == KT - 1),
                            )
                for mi, m in enumerate(chunk):
                    t = outp.tile([P, H], F16, name="osb")
                    for n in range(NT):
                        nsl = slice(n * NFREE, (n + 1) * NFREE)
                        # out = bias * top_p + psum   (single DVE op)
                        nc.vector.scalar_tensor_tensor(
                            t[:, nsl], bias_sb[:, nsl],
                            scale_sb[:, m:m + 1], ps[m, n][:],
                            op0=ALU.mult, op1=ALU.add,
                        )
                        if len(chunk) == 1:
                            # tail chunks: ship each half as soon as its
                            # eviction lands
                            eng = nc.sync if n == 0 else nc.scalar
                            eng.dma_start(out_d[m][:, nsl], t[:, nsl])
                    if len(chunk) > 1:
                        eng = nc.sync if mi % 2 == 0 else nc.scalar
                        eng.dma_start(out_d[m], t[:])

    nc.compile()
    return nc


def kernel(input, gate, W, b):
    from concourse import bass_utils

    input = np.ascontiguousarray(input, dtype=np.float32)
    gate = np.ascontiguousarray(gate, dtype=np.float32)
    W = np.ascontiguousarray(W, dtype=np.float32)
    b = np.ascontiguousarray(b, dtype=np.float32)

    # ---- router (host): top-1 expert + its softmax probability ----
    g = gate.astype(np.float64)
    gm = g.max(axis=1, keepdims=True)
    top_p = (1.0 / np.exp(g - gm).sum(axis=1)).astype(np.float32)
    e_t = np.argmax(gate, axis=1)

    counts = np.bincount(e_t, minlength=E)
    order = np.argsort(e_t, kind="stable")
    starts = np.zeros(E + 1, dtype=np.int64)
    np.cumsum(counts, out=starts[1:])

    C = max(P, int(-(-counts.max() // P)) * P)
    MT = C // P

    if C not in _BUILD_CACHE:
        _BUILD_CACHE[C] = _build(C)
    nc = _BUILD_CACHE[C]

    in_maps = []
    ids_per_e = []
    for e in range(E):
        ids = order[starts[e]:starts[e + 1]]
        ids_per_e.append(ids)
        n_e = len(ids)

        xt = np.zeros((KT, P, C), dtype=np.float16)
        # tokens pre-scaled by their gate probability
        xt.reshape(H, C)[:, :n_e] = (input[ids].T * top_p[ids][None, :]).astype(np.float16)

        scale = np.zeros((MT, P), dtype=np.float32)
        scale.reshape(C)[:n_e] = top_p[ids]
        scale = np.ascontiguousarray(scale.T)

        in_maps.append({
            "xt": xt,
            "w": W[e].astype(np.float16).reshape(KT, P, H),
            "bias": np.ascontiguousarray(np.broadcast_to(b[e].astype(np.float16), (P, H))),
            "scale": scale,
        })

    res = bass_utils.run_bass_kernel_spmd(nc, in_maps,
                                          core_ids=list(range(N_CORES)))

    out = np.empty((T, H), dtype=np.float32)
    for e in range(E):
        ids = ids_per_e[e]
        out[ids] = res.results[e]["out"].reshape(C, H)[:len(ids)].astype(np.float32)
    return out


# revision 11
# speedup vs baseline: 1.0702x; 1.0286x over previous
"""Mixture-of-Experts (top-1 routing) Trainium2 kernel.

Strategy (expert-parallel with one overflow slot, per sharding hint):
 - Router (softmax / argmax / top-prob) evaluated on host — 8192x8, i.e.
   0.002% of the FLOPs; its cost is dispatch bookkeeping.
 - Core e owns expert e.  The first MT-1 m-tiles of a core hold tokens of
   its primary expert; the last m-tile is an overflow slot (own-expert
   overflow, or up to 128 tokens of one overloaded foreign expert, using
   the core's secondary weight tensor).  This balances the per-core token
   capacity C = MT*128 below the max expert load, shrinking both the
   TensorEngine work and the DMA volume on every core.
 - Each core runs a dense [C,1024] @ [1024,1024] GEMM on the TensorEngine
   with fp16 operands and fp32 PSUM accumulation (~4.5e-4 max rel err
   end-to-end).  PSUM eviction fuses the bias in a single DVE op per
   tile: out = (bias * top_p) + psum.
 - 8 dummy matmuls on zeroed tiles run during the DMA head phase to open
   the PE HAM clock gate (1.2 -> 2.4 GHz) before real work arrives.
 - Host scatters the compact per-core outputs back to token order
   (the "second all-to-all" / unshard step).
"""

import numpy as np

T, H, E = 8192, 1024, 8
N_CORES = 8
P = 128
KT = H // P          # 8 contraction tiles
NFREE = 512          # matmul moving free dim (one PSUM bank of fp32)
NT = H // NFREE      # 2 output column tiles

_BUILD_CACHE = {}


def _build(MT):
    """Build the SPMD Bass module for MT m-tiles per core (C = MT*128).

    m-tiles 0..MT-2 use the primary weights (w / bias); m-tile MT-1 uses
    the secondary weights (w2 / bias2) — the overflow slot.
    """
    import concourse.mybir as mybir
    import concourse.tile as tile
    from concourse import bacc

    C = MT * P
    DT = mybir.dt.float16    # half-precision I/O, full-rate matmul
    F32 = mybir.dt.float32
    F16 = mybir.dt.float16
    ALU = mybir.AluOpType

    nc = bacc.Bacc("TRN2", target_bir_lowering=False, debug=False,
                   num_devices=N_CORES)

    xt_d = nc.dram_tensor("xt", [KT, P, C], DT, kind="ExternalInput").ap()
    w_d = nc.dram_tensor("w", [KT, P, H], DT, kind="ExternalInput").ap()
    w2_d = nc.dram_tensor("w2", [KT, P, H], DT, kind="ExternalInput").ap()
    bias_d = nc.dram_tensor("bias", [P, H], F16, kind="ExternalInput").ap()
    bias2_d = nc.dram_tensor("bias2", [P, H], F16, kind="ExternalInput").ap()
    scale_d = nc.dram_tensor("scale", [P, MT], F32, kind="ExternalInput").ap()
    out_d = nc.dram_tensor("out", [MT, P, H], F16, kind="ExternalOutput").ap()

    CH = 4  # m-tiles per chunk (4m x 2n = 8 PSUM banks)
    m_chunks = [list(range(s, min(s + CH, MT))) for s in range(0, MT, CH)]
    # split the final chunk into single m-tiles so the kernel tail
    # (eviction + out-DMA after the last matmul) is as short as possible
    if len(m_chunks) > 1 and len(m_chunks[-1]) > 1:
        last = m_chunks.pop()
        m_chunks.extend([m] for m in last)

    with tile.TileContext(nc) as tc:
        with (
            tc.tile_pool(name="ins", bufs=1) as ins,
            tc.tile_pool(name="psum", bufs=1, space="PSUM") as psum_pool,
            tc.tile_pool(name="outp", bufs=4) as outp,
        ):
            xt_sb = [ins.tile([P, C], DT, name=f"xt{k}") for k in range(KT)]
            w_sb = [ins.tile([P, H], DT, name=f"w{k}") for k in range(KT)]
            w2_sb = [ins.tile([P, H], DT, name=f"w2_{k}") for k in range(KT)]
            bias_sb = ins.tile([P, H], F16, name="bias")
            bias2_sb = ins.tile([P, H], F16, name="bias2")
            scale_sb = ins.tile([P, MT], F32, name="scale")

            # PE warm-up: 8 dummy matmuls on a zeroed tile run during the
            # DMA head phase so the HAM clock-gate opens (1.2->2.4 GHz)
            # before the first real matmul issues.  The warm-up PSUM tile
            # shares the ps0_0 slot: its last write completes long before
            # chunk 0's data arrives, so the WAW hand-off is free.
            wz = ins.tile([P, P + NFREE], DT, name="wz")
            nc.gpsimd.memset(wz[:], 0)
            warm_ps = psum_pool.tile([P, NFREE], F32, name="ps0_0")
            for _ in range(8):
                nc.tensor.matmul(warm_ps[:], wz[:, :P], wz[:, P:],
                                 start=True, stop=True)

            # tiny scale/bias inputs ride the SWDGE GpSimd queue, keeping
            # both HWDGE queues free for the k-major xt / w streams
            nc.gpsimd.dma_start(scale_sb[:], scale_d[:])
            nc.gpsimd.dma_start(bias_sb[:], bias_d[:])
            nc.gpsimd.dma_start(bias2_sb[:], bias2_d[:])
            for k in range(KT):
                nc.sync.dma_start(xt_sb[k][:], xt_d[k])
                nc.scalar.dma_start(w_sb[k][:], w_d[k])
            # secondary weights are only needed by the last m-tile — stream
            # them after the primary inputs, split across both queues
            for k in range(KT):
                eng = nc.sync if k % 2 == 0 else nc.scalar
                eng.dma_start(w2_sb[k][:], w2_d[k])

            for chunk in m_chunks:
                ps = {}
                for m in chunk:
                    for n in range(NT):
                        ps[m, n] = psum_pool.tile([P, NFREE], F32,
                                                  name=f"ps{m % CH}_{n}")
                for k in range(KT):
                    for m in chunk:
                        wk = w2_sb[k] if m == MT - 1 else w_sb[k]
                        for n in range(NT):
                            nc.tensor.matmul(
                                ps[m, n][:],
                                xt_sb[k][:, m * P:(m + 1) * P],
                                wk[:, n * NFREE:(n + 1) * NFREE],
                                start=(k == 0), stop=(k == KT - 1),
                            )
                for mi, m in enumerate(chunk):
                    bsb = bias2_sb if m == MT - 1 else bias_sb
                    t = outp.tile([P, H], F16, name="osb")
                    for n in range(NT):
                        nsl = slice(n * NFREE, (n + 1) * NFREE)
                        # out = bias * top_p + psum   (single DVE op)
                        nc.vector.scalar_tensor_tensor(
                            t[:, nsl], bsb[:, nsl],
                            scale_sb[:, m:m + 1], ps[m, n][:],
                            op0=ALU.mult, op1=ALU.add,
                        )
                        if len(chunk) == 1:
                            # tail chunks: ship each half as soon as its
                            # eviction lands
                            eng = nc.sync if n == 0 else nc.scalar
                            eng.dma_start(out_d[m][:, nsl], t[:, nsl])
                    if len(chunk) > 1:
                        eng = nc.sync if mi % 2 == 0 else nc.scalar
                        eng.dma_start(out_d[m], t[:])

    nc.compile()
    return nc


def _plan(counts):
    """Pick MT and the overflow assignment.

    Returns (MT, sec_expert[core], sec_ids_slice[core]) where each core's
    secondary (overflow) m-tile holds up to 128 tokens: its own expert's
    overflow beyond (MT-1)*128, or one foreign chunk of an overloaded
    expert.  Feasibility: every expert's tokens beyond MT*128 must fit in
    128-token chunks on cores whose own expert fits in (MT-1)*128.
    """
    mt_hi = max(1, int(-(-counts.max() // P)))          # plain expert-parallel
    mt_lo = max(1, int(-(-(counts.sum() // E) // P)))
    for MT in range(mt_lo, mt_hi + 1):
        prim = (MT - 1) * P
        ext = [max(0, int(c) - MT * P) for c in counts]
        slots_needed = sum(-(-x // P) for x in ext)
        free = [e for e in range(E) if counts[e] <= prim]
        if slots_needed <= len(free):
            return MT, prim, ext, free
    MT = mt_hi
    prim = (MT - 1) * P
    return MT, prim, [0] * E, []


def kernel(input, gate, W, b):
    from concourse import bass_utils

    input = np.ascontiguousarray(input, dtype=np.float32)
    gate = np.ascontiguousarray(gate, dtype=np.float32)
    W = np.ascontiguousarray(W, dtype=np.float32)
    b = np.ascontiguousarray(b, dtype=np.float32)

    # ---- router (host): top-1 expert + its softmax probability ----
    g = gate.astype(np.float64)
    gm = g.max(axis=1, keepdims=True)
    top_p = (1.0 / np.exp(g - gm).sum(axis=1)).astype(np.float32)
    e_t = np.argmax(gate, axis=1)

    counts = np.bincount(e_t, minlength=E)
    order = np.argsort(e_t, kind="stable")
    starts = np.zeros(E + 1, dtype=np.int64)
    np.cumsum(counts, out=starts[1:])
    ids_of = [order[starts[e]:starts[e + 1]] for e in range(E)]

    MT, prim, ext, free = _plan(counts)
    C = MT * P

    # Per-core token layout: primary expert tokens in cols [0, prim) and
    # own-overflow (up to 128) in the overflow slot; foreign chunks of
    # overloaded experts go to free cores' overflow slots.
    core_prim_ids = []      # ids in the primary region
    core_sec_ids = []       # ids in the overflow m-tile
    core_sec_expert = []
    for e in range(E):
        ids = ids_of[e]
        n_own_prim = min(len(ids), prim)
        n_own_sec = min(P, max(0, len(ids) - prim))
        core_prim_ids.append(ids[:n_own_prim])
        core_sec_ids.append(ids[n_own_prim:n_own_prim + n_own_sec])
        core_sec_expert.append(e)
    # distribute external overflow chunks to free cores
    free_iter = iter(free)
    for e in range(E):
        leftover = ids_of[e][prim + P:] if len(ids_of[e]) > prim + P else []
        o = 0
        while o < len(leftover):
            host = next(free_iter)
            chunk = leftover[o:o + P]
            core_sec_ids[host] = chunk
            core_sec_expert[host] = e
            o += P

    W16 = W.astype(np.float16)
    b16 = b.astype(np.float16)

    if MT not in _BUILD_CACHE:
        _BUILD_CACHE[MT] = _build(MT)
    nc = _BUILD_CACHE[MT]

    in_maps = []
    for e in range(E):
        pi, si, se = core_prim_ids[e], core_sec_ids[e], core_sec_expert[e]
        n_p, n_s = len(pi), len(si)

        xt = np.zeros((KT, P, C), dtype=np.float16)
        xtf = xt.reshape(H, C)
        if n_p:
            xtf[:, :n_p] = (input[pi].T * top_p[pi][None, :]).astype(np.float16)
        if n_s:
            xtf[:, prim:prim + n_s] = (input[si].T * top_p[si][None, :]).astype(np.float16)

        scale = np.zeros((MT, P), dtype=np.float32)
        sf = scale.reshape(C)
        sf[:n_p] = top_p[pi]
        sf[prim:prim + n_s] = top_p[si]
        scale = np.ascontiguousarray(scale.T)

        in_maps.append({
            "xt": xt,
            "w": W16[e].reshape(KT, P, H),
            "w2": W16[se].reshape(KT, P, H),
            "bias": np.ascontiguousarray(np.broadcast_to(b16[e], (P, H))),
            "bias2": np.ascontiguousarray(np.broadcast_to(b16[se], (P, H))),
            "scale": scale,
        })

    res = bass_utils.run_bass_kernel_spmd(nc, in_maps,
                                          core_ids=list(range(N_CORES)))

    out = np.empty((T, H), dtype=np.float32)
    for e in range(E):
        r = res.results[e]["out"].reshape(C, H)
        pi, si = core_prim_ids[e], core_sec_ids[e]
        if len(pi):
            out[pi] = r[:len(pi)].astype(np.float32)
        if len(si):
            out[si] = r[prim:prim + len(si)].astype(np.float32)
    return out
